# revision 33
# baseline (speedup 1.0000x reference)
"""Trainium2 Bass kernel for nn_CompensationAlignmentModule.

Strategy (8 NeuronCores, SPMD):
  - Data-parallel over flat tokens N=8192 -> 1024 rows per core.
  - bf16 matmul operands everywhere (fp32 PSUM accumulation); LN/l2norm/
    softmax/loss math in fp32.
  - Transposes into the [h-on-partitions] matmul layout go through bf16
    DRAM round-trips using the DMA crossbar transpose (no PE time).
  - vb is computed first, transposed, and AllGathered; all weights are
    preloaded and xnT is prepared early so the scale/bias-head GEMMs (pure
    PE work) execute underneath the collective.
  - Each core computes its 1024x8192 block of contrastive logits against
    the full vb, exporting per-row sum(exp), per-column partial sums and
    diagonal candidates.
  - Token-distribution loss per-core on the sampled rows it owns (one-hot
    matmul select, top-64 via DVE max8 + match_replace, prototype via
    masked-softmax matmul); emitted before the logits phase so its serial
    chain hides under PE-bound work.
  - Host: input slicing/layout, final O(N) reductions (log/mean) only.
"""

import contextlib

import ml_dtypes
import numpy as np

import concourse.bass as bass
import concourse.mybir as mybir
import concourse.tile as tile
from concourse.bass import ds, ts
from concourse.bass_utils import run_bass_kernel_spmd
from concourse.masks import make_identity

F32 = mybir.dt.float32
BF16 = mybir.dt.bfloat16
AF = mybir.ActivationFunctionType
ALU = mybir.AluOpType
AX = mybir.AxisListType

NCORES = 8
H = 768
HS = H // 128          # 6 h-subtiles
NL = 1024              # rows per core
RT = NL // 128         # 8 row tiles
NGLOB = NCORES * NL    # 8192
POOLN = 1024           # token pool size
SCAP = 128             # per-core sampled-row capacity
KTOP = 64
TEMP = 0.1
ITEMP = 1.0 / TEMP
LIM = 0.25
MW = 0.1
EPS = 1e-5
MINV = -1.0e30
P = 128


def _split_drain_waits(nc):
    """This container's walrus accepts at most ONE sync wait per instruction,
    while Tile's add_semaphores pass attaches several. Move extra waits onto
    NoOp instructions inserted right before (same engine, serial execution,
    so blocking semantics are preserved)."""
    for f in nc.m.functions:
        for bb in f.blocks:
            out = []
            changed = False
            for inst in bb.instructions:
                si = inst.sync_info
                if si is not None and len(si.on_wait) > 1:
                    waits = list(si.on_wait)
                    for k, w in enumerate(waits[:-1]):
                        nop = mybir.InstNoOp(name=f"{inst.name}-w{k}", ins=[], outs=[])
                        nop.engine = inst.engine
                        nop.sync_info = mybir.SyncInfo(on_update=[], on_wait=[w])
                        out.append(nop)
                        nc.register_instruction(nop, overwrite=True)
                    si.on_wait = [waits[-1]]
                    changed = True
                out.append(inst)
            if changed:
                bb.instructions = out


def _ln_standardize(nc, pool, src_ap, dst_ap, eps_t):
    """dst = (src - mean)/sqrt(var+eps) rowwise over H ([128, H] tiles).
    dst may be bf16 (fused cast). Uses bn_stats (Welford) for mean/var."""
    sg = src_ap.rearrange("p (n s) -> p n s", s=256)
    stats = pool.tile([P, 3, 6], F32, tag="bnst")
    for g in range(3):
        nc.vector.bn_stats(out=stats[:, g, :], in_=sg[:, g, :])
    mv = pool.tile([P, 2], F32, tag="bnmv")
    nc.vector.bn_aggr(out=mv, in_=stats)
    sd = pool.tile([P, 1], F32, tag="ln_sd")
    nc.scalar.activation(sd, mv[:, 1:2], AF.Sqrt, bias=eps_t)
    rr = pool.tile([P, 1], F32, tag="ln_rr")
    nc.vector.reciprocal(rr, sd)
    nc.vector.tensor_scalar(
        dst_ap, src_ap, mv[:, 0:1], rr, op0=ALU.subtract, op1=ALU.mult
    )


def _l2norm_rows(nc, pool, v_ap, out_ap=None):
    """l2-normalize rows of v_ap [128, H]; result lands in out_ap (may be
    bf16 -> fused cast) or in place."""
    sq = pool.tile([P, H], F32, tag="ln_sq")
    ss = pool.tile([P, 1], F32, tag="ln_ss")
    nc.scalar.activation(sq, v_ap, AF.Square, accum_out=ss)
    nrm = pool.tile([P, 1], F32, tag="ln_m")
    nc.scalar.activation(nrm, ss, AF.Sqrt)
    nc.vector.tensor_scalar_max(nrm, nrm, 1e-12)
    rr = pool.tile([P, 1], F32, tag="ln_rr")
    nc.vector.reciprocal(rr, nrm)
    nc.vector.tensor_scalar_mul(
        out_ap if out_ap is not None else v_ap, v_ap, rr
    )


def _build_nc(ln_trivial):
    nc = bass.Bass(num_devices=NCORES, name="comp_align")

    # ---------------- DRAM I/O ----------------
    def inp(name, shape, dt=F32):
        return nc.dram_tensor(name, shape, dt, kind="ExternalInput")

    x_d = inp("x", [NL, H])
    na_d = inp("na", [NL, H])
    nb_d = inp("nb_", [NL, H])
    w_d = {k: inp(k, [H, H], BF16)
           for k in ("w1s", "w2s", "w1b", "w2b", "w1c", "w2c")}
    b1_d = {k: inp(k, [P, HS]) for k in ("b1s", "b1b", "b1c")}
    b2_d = {k: inp(k, [P, H]) for k in ("b2s", "b2b", "b2c")}
    g_d = {k: inp(k, [P, HS]) for k in ("gs", "bts", "gb", "btb", "gc", "btc")}
    te_d = inp("te", [POOLN, H])
    tenT_d = inp("tenT", [H, POOLN])
    ohT_d = inp("ohT", [NL, SCAP])
    val_d = inp("val", [SCAP, 1])

    alig_o = nc.dram_tensor("alig_o", [NL, H], F32, kind="ExternalOutput")
    scal_o = nc.dram_tensor("scal_o", [NL, H], F32, kind="ExternalOutput")
    bias_o = nc.dram_tensor("bias_o", [NL, H], F32, kind="ExternalOutput")
    dcand_o = nc.dram_tensor("dcand_o", [P, RT, NCORES], F32, kind="ExternalOutput")
    rsum_o = nc.dram_tensor("rsum_o", [P, RT], F32, kind="ExternalOutput")
    csum_o = nc.dram_tensor("csum_o", [2 * NCORES, 512], F32, kind="ExternalOutput")
    reg_o = nc.dram_tensor("reg_o", [P, 2], F32, kind="ExternalOutput")
    afm_o = nc.dram_tensor("afm_o", [SCAP, H], F32, kind="ExternalOutput")
    prot_o = nc.dram_tensor("prot_o", [SCAP, H], F32, kind="ExternalOutput")

    with tile.TileContext(nc) as tc:
        with contextlib.ExitStack() as ctx:
            # ------------ long-lived pools ------------
            const = ctx.enter_context(tc.tile_pool(name="const", bufs=1))
            longp = ctx.enter_context(tc.tile_pool(name="longp", bufs=1))
            accp = ctx.enter_context(tc.tile_pool(name="accp", bufs=1))
            dram = ctx.enter_context(tc.tile_pool(name="dram", bufs=1, space="DRAM"))

            identity = const.tile([P, P], F32)
            make_identity(nc, identity)
            ones_col = const.tile([P, 1], F32)
            nc.vector.memset(ones_col, 1.0)
            eps_t = const.tile([P, 1], F32)
            nc.vector.memset(eps_t, EPS)
            b1_sb = {k: const.tile([P, HS], F32, name=f"sb_{k}") for k in b1_d}
            for k in b1_d:
                nc.sync.dma_start(b1_sb[k][:], b1_d[k][:])
            b2_sb = {k: const.tile([P, H], F32, name=f"sb_{k}") for k in b2_d}
            for k in b2_d:
                nc.sync.dma_start(b2_sb[k][:], b2_d[k][:])
            g_sb = {k: const.tile([P, HS], F32, name=f"sb_{k}") for k in g_d}
            for k in g_d:
                nc.sync.dma_start(g_sb[k][:], g_d[k][:])
            val_sb = const.tile([SCAP, 1], F32)
            nc.sync.dma_start(val_sb[:], val_d[:])

            # x shard stays resident; vaT persists into the logits phase
            x_sb = longp.tile([P, RT, H], F32)
            nc.sync.dma_start(x_sb[:], x_d.rearrange("(rt p) h -> p rt h", p=P))
            vaT = longp.tile([P, HS, NL], BF16)

            # accumulators
            rs_parts = accp.tile([P, RT, 16], F32)
            dcand = accp.tile([P, RT, NCORES], F32)
            racc_s = accp.tile([P, 16], F32)
            racc_b = accp.tile([P, 16], F32)

            # DRAM scratch
            ag_in = [dram.tile([HS, P, 512], BF16, name=f"ag_in{h}")
                     for h in range(2)]
            ag_out = [
                dram.tile([NCORES, HS, P, 512], BF16, addr_space="Shared",
                          name=f"ag_out{h}")
                for h in range(2)
            ]
            alig_d = dram.tile([NL, H], F32)
            xt_d = {k: dram.tile([NL, H], BF16, name=f"xt_{k}")
                    for k in ("xn", "xa", "xb", "va", "vb")}

            def std_store_transpose(noise_dram, xdram, dstT, lnp,
                                    ln_g, ln_b):
                """Standardize x (+ optional noise) per row tile, cast to
                bf16, store to DRAM, then two half DMA-crossbar transposes
                into dstT [128, HS, NL] bf16 (halves so downstream GEMMs can
                start after 4 row tiles)."""
                for rt in range(RT):
                    if noise_dram is not None:
                        xa_t = lnp.tile([P, H], F32, tag="xa")
                        nc.sync.dma_start(xa_t[:], noise_dram[ds(rt * P, P), :])
                        nc.vector.tensor_add(xa_t, x_sb[:, rt, :], xa_t)
                        src_ap = xa_t[:]
                    else:
                        src_ap = x_sb[:, rt, :]
                    xab = lnp.tile([P, H], BF16, tag="xab")
                    _ln_standardize(nc, lnp, src_ap, xab[:], eps_t)
                    nc.sync.dma_start(xdram[ds(rt * P, P), :], xab[:])
                    if rt in (RT // 2 - 1, RT - 1):
                        hb = 0 if rt < RT // 2 else 1
                        nc.sync.dma_start_transpose(
                            dstT[:, :, ts(hb, 512)], xdram[ds(hb * 512, 512), :]
                        )
                if not ln_trivial:
                    for hs in range(HS):
                        nc.vector.tensor_scalar(
                            dstT[:, hs, :], dstT[:, hs, :],
                            ln_g[:, hs : hs + 1], ln_b[:, hs : hs + 1],
                            op0=ALU.mult, op1=ALU.add,
                        )

            with contextlib.ExitStack() as hctx:
                wall = hctx.enter_context(tc.tile_pool(name="wall", bufs=1))
                bigp = hctx.enter_context(tc.tile_pool(name="bigp", bufs=4))
                lnp = hctx.enter_context(tc.tile_pool(name="lnp", bufs=2))
                epp = hctx.enter_context(tc.tile_pool(name="epp", bufs=3))
                pscon = hctx.enter_context(
                    tc.tile_pool(name="pscon", bufs=3, space="PSUM")
                )
                psa = hctx.enter_context(
                    tc.tile_pool(name="psa", bufs=3, space="PSUM")
                )

                # preload every weight up front (keeps the DMA queues free
                # while the AllGather runs)
                w_sb = {}
                for k in ("w1c", "w2c", "w1s", "w1b", "w2s", "w2b"):
                    w_sb[k] = wall.tile([P, HS, H], BF16, name=f"w_{k}")
                    nc.sync.dma_start(
                        w_sb[k][:], w_d[k].rearrange("(ks p) o -> p ks o", p=P)
                    )

                def first_gemm(w1_sb, lnT, g1T, b1, psum):
                    for mt in range(HS):
                        for nb in range(2):
                            ps = psum.tile([P, 512], F32, tag="gemm")
                            for ks in range(HS):
                                nc.tensor.matmul(
                                    ps,
                                    w1_sb[:, ks, ts(mt, P)],
                                    lnT[:, ks, ts(nb, 512)],
                                    start=(ks == 0),
                                    stop=(ks == HS - 1),
                                )
                            nc.scalar.activation(
                                g1T[:, mt, ts(nb, 512)], ps, AF.Gelu,
                                bias=b1[:, mt : mt + 1],
                            )

                def con_second(which, g1T, ag):
                    """second GEMM of the contrastive head; l2-normalized bf16
                    rows land in xt_d[which]; at each half boundary kick the
                    DMA transpose (and, for vb, the AllGather half)."""
                    for rt in range(RT):
                        vr = lnp.tile([P, H], F32, tag="xa")
                        for nb2, width in ((0, 512), (1, 256)):
                            ps = pscon.tile([P, 512], F32, tag="gemm")
                            pw = ps[:, :width]
                            for ks in range(HS):
                                nc.tensor.matmul(
                                    pw,
                                    g1T[:, ks, ts(rt, P)],
                                    w_sb["w2c"][:, ks, ds(nb2 * 512, width)],
                                    start=(ks == 0),
                                    stop=(ks == HS - 1),
                                )
                            nc.vector.tensor_add(
                                vr[:, ds(nb2 * 512, width)], pw,
                                b2_sb["b2c"][:, ds(nb2 * 512, width)],
                            )
                        vb_ = lnp.tile([P, H], BF16, tag="xab")
                        _l2norm_rows(nc, lnp, vr[:], vb_[:])
                        nc.sync.dma_start(xt_d[which][ds(rt * P, P), :], vb_[:])
                        if rt in (RT // 2 - 1, RT - 1):
                            hb = 0 if rt < RT // 2 else 1
                            vT = vaT if which == "va" else vbT
                            nc.sync.dma_start_transpose(
                                vT[:, :, ts(hb, 512)],
                                xt_d[which][ds(hb * 512, 512), :],
                            )
                            if ag:
                                nc.sync.dma_start(
                                    ag_in[hb][:].rearrange("ks p i -> p ks i"),
                                    vT[:, :, ts(hb, 512)],
                                )
                                nc.gpsimd.collective_compute(
                                    "AllGather",
                                    ALU.bypass,
                                    replica_groups=[list(range(NCORES))],
                                    ins=[ag_in[hb][:].opt()],
                                    outs=[ag_out[hb][:].opt()],
                                )

                # ---- all LN chains first (DVE/ACT/DMA work, PE-free) ----
                xbnT = bigp.tile([P, HS, NL], BF16, tag="bigT", name="xbnT")
                std_store_transpose(nb_d, xt_d["xb"], xbnT, lnp,
                                    g_sb["gc"], g_sb["btc"])
                xanT = bigp.tile([P, HS, NL], BF16, tag="bigT", name="xanT")
                std_store_transpose(na_d, xt_d["xa"], xanT, lnp,
                                    g_sb["gc"], g_sb["btc"])
                xnT = bigp.tile([P, HS, NL], BF16, tag="bigT", name="xnT")
                std_store_transpose(None, xt_d["xn"], xnT, lnp,
                                    g_sb["gs"], g_sb["bts"])
                if ln_trivial:
                    lnT = {"s": xnT, "b": xnT}
                else:
                    lnT = {"s": xnT,
                           "b": bigp.tile([P, HS, NL], BF16, tag="bigT",
                                          name="lnT_b")}
                    std_store_transpose(None, xt_d["xn"], lnT["b"], lnp,
                                        g_sb["gb"], g_sb["btb"])

                # ---- view b head -> split AllGather (earliest possible) ----
                g1T_vb = bigp.tile([P, HS, NL], BF16, tag="bigT", name="g1T_vb")
                first_gemm(w_sb["w1c"], xbnT, g1T_vb, b1_sb["b1c"], pscon)
                vbT = bigp.tile([P, HS, NL], BF16, tag="bigT", name="vbT")
                con_second("vb", g1T_vb, ag=True)

                # ---- view a head (fills PE while the AllGather flies) ----
                g1T_va = bigp.tile([P, HS, NL], BF16, tag="bigT", name="g1T_va")
                first_gemm(w_sb["w1c"], xanT, g1T_va, b1_sb["b1c"], pscon)
                con_second("va", g1T_va, ag=False)

                # ---- scale & bias heads ----
                g1T_s = bigp.tile([P, HS, NL], BF16, tag="bigT", name="g1T_s")
                first_gemm(w_sb["w1s"], lnT["s"], g1T_s, b1_sb["b1s"], psa)
                g1T_b = bigp.tile([P, HS, NL], BF16, tag="bigT", name="g1T_b")
                first_gemm(w_sb["w1b"], lnT["b"], g1T_b, b1_sb["b1b"], psa)

                for rt in range(RT):
                    for nb2, width in ((0, 512), (1, 256)):
                        sl = ds(nb2 * 512, width)
                        col = rt * 2 + nb2
                        # scale head tile
                        ps_s = psa.tile([P, 512], F32, tag="gemm")
                        for ks in range(HS):
                            nc.tensor.matmul(
                                ps_s[:, :width],
                                g1T_s[:, ks, ts(rt, P)],
                                w_sb["w2s"][:, ks, sl],
                                start=(ks == 0), stop=(ks == HS - 1),
                            )
                        h1 = epp.tile([P, 512], F32, tag="ep_h1")
                        nc.vector.tensor_add(
                            h1[:, :width], ps_s[:, :width], b2_sb["b2s"][:, sl]
                        )
                        th_s = epp.tile([P, 512], F32, tag="ep_ths")
                        nc.scalar.activation(th_s[:, :width], h1[:, :width], AF.Tanh)
                        sqs = epp.tile([P, 512], F32, tag="ep_h1")
                        nc.scalar.activation(
                            sqs[:, :width], th_s[:, :width], AF.Square,
                            accum_out=racc_s[:, col : col + 1],
                        )
                        sc = epp.tile([P, 512], F32, tag="ep_sc")
                        nc.vector.tensor_scalar(
                            sc[:, :width], th_s[:, :width],
                            LIM, 1.0, op0=ALU.mult, op1=ALU.add,
                        )
                        nc.sync.dma_start(scal_o[ds(rt * P, P), sl], sc[:, :width])
                        # bias head tile
                        ps_b = psa.tile([P, 512], F32, tag="gemm")
                        for ks in range(HS):
                            nc.tensor.matmul(
                                ps_b[:, :width],
                                g1T_b[:, ks, ts(rt, P)],
                                w_sb["w2b"][:, ks, sl],
                                start=(ks == 0), stop=(ks == HS - 1),
                            )
                        h2 = epp.tile([P, 512], F32, tag="ep_h1")
                        nc.vector.tensor_add(
                            h2[:, :width], ps_b[:, :width], b2_sb["b2b"][:, sl]
                        )
                        th_b = epp.tile([P, 512], F32, tag="ep_thb")
                        nc.scalar.activation(th_b[:, :width], h2[:, :width], AF.Tanh)
                        sqb = epp.tile([P, 512], F32, tag="ep_h1")
                        nc.scalar.activation(
                            sqb[:, :width], th_b[:, :width], AF.Square,
                            accum_out=racc_b[:, col : col + 1],
                        )
                        bi = epp.tile([P, 512], F32, tag="ep_bi")
                        nc.vector.tensor_scalar_mul(bi[:, :width], th_b[:, :width], LIM)
                        nc.sync.dma_start(bias_o[ds(rt * P, P), sl], bi[:, :width])
                        # aligned = scale * x + bias
                        al = epp.tile([P, 512], F32, tag="ep_al")
                        nc.vector.tensor_mul(
                            al[:, :width], sc[:, :width], x_sb[:, rt, sl]
                        )
                        nc.vector.tensor_add(al[:, :width], al[:, :width], bi[:, :width])
                        nc.sync.dma_start(alig_o[ds(rt * P, P), sl], al[:, :width])
                        nc.sync.dma_start(alig_d[ds(rt * P, P), sl], al[:, :width])

                rs1 = lnp.tile([P, 1], F32, tag="reg1")
                nc.vector.reduce_sum(out=rs1, in_=racc_s[:], axis=AX.X)
                nc.sync.dma_start(reg_o[:, 0:1], rs1[:])
                rs2 = lnp.tile([P, 1], F32, tag="reg2")
                nc.vector.reduce_sum(out=rs2, in_=racc_b[:], axis=AX.X)
                nc.sync.dma_start(reg_o[:, 1:2], rs2[:])


            # =========================================================
            # Phases C + D together: the serial token-distribution chain
            # (C) hides under the PE-bound logits phase (D).
            # =========================================================
            with contextlib.ExitStack() as dctx:
                tokp = dctx.enter_context(tc.tile_pool(name="tokp", bufs=1))
                tok2 = dctx.enter_context(tc.tile_pool(name="tok2", bufs=2))
                psc = dctx.enter_context(
                    tc.tile_pool(name="ps_c", bufs=2, space="PSUM")
                )
                pstc = dctx.enter_context(
                    tc.tile_pool(name="pst_c", bufs=1, space="PSUM")
                )
                kxn = dctx.enter_context(tc.tile_pool(name="kxn", bufs=3))
                expp = dctx.enter_context(tc.tile_pool(name="expp", bufs=4))
                caccp = dctx.enter_context(tc.tile_pool(name="caccp", bufs=2))
                csump = dctx.enter_context(tc.tile_pool(name="csump", bufs=2))
                junkp = dctx.enter_context(tc.tile_pool(name="junkp", bufs=2))
                psd = dctx.enter_context(
                    tc.tile_pool(name="ps_d", bufs=3, space="PSUM")
                )
                psd1 = dctx.enter_context(
                    tc.tile_pool(name="ps_d1", bufs=1, space="PSUM")
                )

                # ---------------- Phase C ----------------
                ohT_sb = tokp.tile([P, RT, SCAP], F32)
                nc.sync.dma_start(
                    ohT_sb[:], ohT_d.rearrange("(ks p) s -> p ks s", p=P)
                )
                af = tokp.tile([P, H], F32)
                for nb2, width in ((0, 512), (1, 256)):
                    ps = psc.tile([P, 512], F32, tag="cgem")
                    pw = ps[:, :width]
                    for ks in range(RT):
                        rhs_t = tok2.tile([P, 512], F32, tag="argh")
                        nc.sync.dma_start(
                            rhs_t[:, :width],
                            alig_d[ds(ks * P, P), ds(nb2 * 512, width)],
                        )
                        nc.tensor.matmul(
                            pw,
                            ohT_sb[:, ks, :],
                            rhs_t[:, :width],
                            start=(ks == 0),
                            stop=(ks == RT - 1),
                        )
                    nc.scalar.copy(af[:, ds(nb2 * 512, width)], pw)

                afn = tokp.tile([P, H], F32)
                nc.vector.tensor_copy(afn[:], af[:])
                _l2norm_rows(nc, tok2, afn[:])

                afnT = tokp.tile([P, HS, SCAP], F32)
                for hs in range(HS):
                    pt = pstc.tile([P, P], F32, tag="tpc")
                    nc.tensor.transpose(pt, afn[:, ts(hs, P)], identity)
                    nc.scalar.copy(afnT[:, hs, :], pt)

                tenT_sb = tokp.tile([P, HS, POOLN], F32)
                nc.sync.dma_start(
                    tenT_sb[:], tenT_d.rearrange("(ks p) j -> p ks j", p=P)
                )
                tl = tokp.tile([P, POOLN], F32)
                for half in range(2):
                    ps = psc.tile([P, 512], F32, tag="cgem")
                    for ks in range(HS):
                        nc.tensor.matmul(
                            ps,
                            afnT[:, ks, :],
                            tenT_sb[:, ks, ts(half, 512)],
                            start=(ks == 0),
                            stop=(ks == HS - 1),
                        )
                    nc.scalar.copy(tl[:, ts(half, 512)], ps)

                # top-64 mask via 8x (max8 + match_replace)
                work = tokp.tile([P, POOLN], F32)
                nc.vector.tensor_copy(work[:], tl[:])
                maxb = tokp.tile([P, 8], F32)
                for _ in range(KTOP // 8):
                    nc.vector.max(out=maxb[:], in_=work[:])
                    nc.vector.match_replace(
                        out=work[:], in_to_replace=maxb[:], in_values=work[:],
                        imm_value=MINV,
                    )
                mask = tokp.tile([P, POOLN], F32)
                nc.vector.tensor_sub(mask[:], tl[:], work[:])
                nc.vector.tensor_scalar_min(mask[:], mask[:], 1.0)

                ew = tokp.tile([P, POOLN], F32)
                nc.scalar.activation(ew[:], tl[:], AF.Exp)
                nc.vector.tensor_mul(ew[:], ew[:], mask[:])
                zz = tokp.tile([P, 1], F32)
                nc.vector.reduce_sum(out=zz, in_=ew[:], axis=AX.X)
                zr = tokp.tile([P, 1], F32)
                nc.vector.reciprocal(zr, zz)

                eT = tokp.tile([P, RT, SCAP], F32)
                for js in range(RT):
                    pt = pstc.tile([P, P], F32, tag="tpc")
                    nc.tensor.transpose(pt, ew[:, ts(js, P)], identity)
                    nc.scalar.copy(eT[:, js, :], pt)

                te_sb = tokp.tile([P, RT, H], F32)
                nc.sync.dma_start(
                    te_sb[:], te_d.rearrange("(js p) o -> p js o", p=P)
                )
                prot = tokp.tile([P, H], F32)
                for nb2, width in ((0, 512), (1, 256)):
                    ps = psc.tile([P, 512], F32, tag="cgem")
                    pw = ps[:, :width]
                    for js in range(RT):
                        nc.tensor.matmul(
                            pw,
                            eT[:, js, :],
                            te_sb[:, js, ds(nb2 * 512, width)],
                            start=(js == 0),
                            stop=(js == RT - 1),
                        )
                    nc.vector.tensor_scalar_mul(prot[:, ds(nb2 * 512, width)], pw, zr)

                nc.vector.tensor_scalar_mul(af[:], af[:], val_sb)
                nc.vector.tensor_scalar_mul(prot[:], prot[:], val_sb)
                nc.sync.dma_start(afm_o[:, :], af[:])
                nc.sync.dma_start(prot_o[:, :], prot[:])

                # ---------------- Phase D ----------------
                for half in range(2):
                    for c2 in range(NCORES):
                        nbi = c2 * 2 + half
                        rhs_t = kxn.tile([P, HS, 512], BF16, tag="kxn")
                        nc.sync.dma_start(
                            rhs_t[:],
                            ag_out[half][c2].rearrange("hs p i -> p hs i"),
                        )
                        cacc = caccp.tile([P, 512], F32, tag="cacc")
                        for rt in range(RT):
                            ps = psd.tile([P, 512], F32, tag="lg")
                            for ks in range(HS):
                                nc.tensor.matmul(
                                    ps,
                                    vaT[:, ks, ts(rt, P)],
                                    rhs_t[:, ks, :],
                                    start=(ks == 0),
                                    stop=(ks == HS - 1),
                                )
                            if half == rt // 4:
                                off = (rt % 4) * P
                                junk = junkp.tile([P, P], F32, tag="junk")
                                nc.vector.tensor_mul(
                                    junk[:], ps[:, ds(off, P)], identity[:]
                                )
                                nc.vector.reduce_sum(
                                    out=dcand[:, rt, c2 : c2 + 1],
                                    in_=junk[:],
                                    axis=AX.X,
                                )
                            et = expp.tile([P, 512], F32, tag="exp")
                            nc.scalar.activation(
                                et[:], ps, AF.Exp, scale=ITEMP,
                                accum_out=rs_parts[:, rt, nbi : nbi + 1],
                            )
                            if rt == 0:
                                nc.vector.tensor_copy(cacc[:], et[:])
                            else:
                                nc.vector.tensor_add(cacc[:], cacc[:], et[:])
                        cps = psd1.tile([1, 512], F32, tag="cs")
                        nc.tensor.matmul(
                            cps, ones_col[:, :], cacc[:], start=True, stop=True
                        )
                        csb = csump.tile([1, 512], F32, tag="csb")
                        nc.scalar.copy(csb[:], cps)
                        nc.sync.dma_start(csum_o[nbi : nbi + 1, :], csb[:, :])

                rsf = csump.tile([P, RT], F32, tag="rsf")
                nc.vector.reduce_sum(out=rsf, in_=rs_parts[:], axis=AX.X)
                nc.sync.dma_start(rsum_o[:, :], rsf[:])
                nc.sync.dma_start(dcand_o[:, :, :], dcand[:])

    _split_drain_waits(nc)
    return nc


_CACHE = {}


def _get_nc(ln_trivial=True):
    key = ("nc", ln_trivial)
    if key not in _CACHE:
        _CACHE[key] = _build_nc(ln_trivial)
    return _CACHE[key]


def _prep_in_maps(inputs):
    f32 = np.float32
    lat = np.ascontiguousarray(np.asarray(inputs["latent"], f32)).reshape(NGLOB, H)
    na = np.ascontiguousarray(np.asarray(inputs["noise_a"], f32))
    nb = np.ascontiguousarray(np.asarray(inputs["noise_b"], f32))
    tem = np.asarray(inputs["token_embedding_matrix"], f32)
    samp = np.asarray(inputs["samp_idx"]).astype(np.int64)
    toki = np.asarray(inputs["tok_idx"]).astype(np.int64)

    te = np.ascontiguousarray(tem[toki])                      # [POOLN, H]
    ten = te / np.maximum(
        np.linalg.norm(te, axis=-1, keepdims=True), 1e-12
    )
    tenT = np.ascontiguousarray((ten * ITEMP).T)              # [H, POOLN] (x 1/TEMP)

    ln_trivial = all(
        np.all(np.asarray(inputs[k]) == 1.0)
        for k in ("scale_ln_g", "bias_ln_g", "con_ln_g")
    ) and all(
        np.all(np.asarray(inputs[k]) == 0.0)
        for k in ("scale_ln_b", "bias_ln_b", "con_ln_b")
    )

    def pk(v):  # [H] vector -> [P, HS] per-partition layout (h = ks*P + p)
        return np.ascontiguousarray(np.asarray(v, f32).reshape(HS, P).T)

    def bc(v):  # [H] vector -> [P, H] broadcast
        return np.ascontiguousarray(
            np.broadcast_to(np.asarray(v, f32)[None, :], (P, H))
        )

    bf16 = ml_dtypes.bfloat16
    common = {
        "w1s": np.ascontiguousarray(np.asarray(inputs["scale_w1"], bf16)),
        "w2s": np.ascontiguousarray(np.asarray(inputs["scale_w2"], bf16)),
        "w1b": np.ascontiguousarray(np.asarray(inputs["bias_w1"], bf16)),
        "w2b": np.ascontiguousarray(np.asarray(inputs["bias_w2"], bf16)),
        "w1c": np.ascontiguousarray(np.asarray(inputs["con_w1"], bf16)),
        "w2c": np.ascontiguousarray(np.asarray(inputs["con_w2"], bf16)),
        "b1s": pk(inputs["scale_b1"]), "b1b": pk(inputs["bias_b1"]),
        "b1c": pk(inputs["con_b1"]),
        "b2s": bc(inputs["scale_b2"]), "b2b": bc(inputs["bias_b2"]),
        "b2c": bc(inputs["con_b2"]),
        "gs": pk(inputs["scale_ln_g"]), "bts": pk(inputs["scale_ln_b"]),
        "gb": pk(inputs["bias_ln_g"]), "btb": pk(inputs["bias_ln_b"]),
        "gc": pk(inputs["con_ln_g"]), "btc": pk(inputs["con_ln_b"]),
        "te": te,
        "tenT": tenT,
    }

    # sampled-row ownership
    slots = [[] for _ in range(NCORES)]
    for g in samp:
        c, loc = int(g) // NL, int(g) % NL
        slots[c].append(loc)
    in_maps = []
    for c in range(NCORES):
        locs = slots[c]
        assert len(locs) <= SCAP, f"core {c} owns {len(locs)} sampled rows > {SCAP}"
        ohT = np.zeros((NL, SCAP), f32)
        val = np.zeros((SCAP, 1), f32)
        for s, loc in enumerate(locs):
            ohT[loc, s] = 1.0
            val[s, 0] = 1.0
        for s in range(len(locs), SCAP):
            ohT[0, s] = 1.0  # dup row 0, masked out by val
        m = dict(common)
        m["x"] = np.ascontiguousarray(lat[c * NL : (c + 1) * NL])
        m["na"] = np.ascontiguousarray(na[c * NL : (c + 1) * NL])
        m["nb_"] = np.ascontiguousarray(nb[c * NL : (c + 1) * NL])
        m["ohT"] = ohT
        m["val"] = val
        in_maps.append(m)
    return in_maps, te, ln_trivial


def _finish(results, te):
    f64 = np.float64
    aligned = np.concatenate([r["alig_o"] for r in results], 0).reshape(8, 1024, H)
    scale = np.concatenate([r["scal_o"] for r in results], 0).reshape(8, 1024, H)
    biast = np.concatenate([r["bias_o"] for r in results], 0).reshape(8, 1024, H)

    # contrastive loss
    diag = np.concatenate(
        [results[c]["dcand_o"][:, :, c].T.reshape(NL) for c in range(NCORES)]
    ) * ITEMP
    rowsum = np.concatenate(
        [results[c]["rsum_o"].T.reshape(NL) for c in range(NCORES)]
    )
    colsum = np.sum([r["csum_o"].reshape(NGLOB) for r in results], axis=0)
    la = -np.mean(diag - np.log(rowsum))
    lb = -np.mean(diag - np.log(colsum))
    con_loss = 0.5 * (la + lb)

    # comp reg
    reg = np.sum([r["reg_o"].sum(0) for r in results], axis=0)
    comp_reg = (LIM * LIM) * (reg[0] + reg[1]) / (NGLOB * H)

    # token distribution
    afm = np.concatenate([r["afm_o"] for r in results], 0).astype(f64)
    prm = np.concatenate([r["prot_o"] for r in results], 0).astype(f64)
    S = 256
    proto_loss = np.sum((afm - prm) ** 2) / (S * H)
    s1 = afm.sum(0)
    s2 = (afm * afm).sum(0)
    af_mean = s1 / S
    af_var = np.maximum(s2 / S - af_mean**2, 0.0)
    af_std = np.sqrt(af_var)
    te64 = te.astype(f64)
    te_mean = te64.mean(0)
    te_std = te64.std(0)
    moment = np.mean((af_mean - te_mean) ** 2) + np.mean((af_std - te_std) ** 2)
    token_dist = proto_loss + MW * moment

    return (
        aligned.astype(np.float32),
        np.array(con_loss, np.float32),
        np.array(token_dist, np.float32),
        np.array(comp_reg, np.float32),
        scale.astype(np.float32),
        biast.astype(np.float32),
    )


def kernel(**inputs):
    in_maps, te, ln_trivial = _prep_in_maps(inputs)
    nc = _get_nc(ln_trivial)
    res = run_bass_kernel_spmd(nc, in_maps, core_ids=list(range(NCORES)))
    return _finish(res.results, te)


# revision 34
# speedup vs baseline: 1.1509x; 1.1509x over previous
"""Trainium2 Bass kernel for nn_CompensationAlignmentModule.

Strategy (8 NeuronCores, SPMD):
  - Data-parallel over flat tokens N=8192 -> 1024 rows per core.
  - bf16 matmul operands everywhere (fp32 PSUM accumulation); LN/l2norm/
    softmax/loss math in fp32.
  - Transposes into the [h-on-partitions] matmul layout go through bf16
    DRAM round-trips using the DMA crossbar transpose (no PE time).
  - vb is computed first, transposed, and AllGathered; all weights are
    preloaded and xnT is prepared early so the scale/bias-head GEMMs (pure
    PE work) execute underneath the collective.
  - Each core computes its 1024x8192 block of contrastive logits against
    the full vb, exporting per-row sum(exp), per-column partial sums and
    diagonal candidates.
  - Token-distribution loss per-core on the sampled rows it owns (one-hot
    matmul select, top-64 via DVE max8 + match_replace, prototype via
    masked-softmax matmul); emitted before the logits phase so its serial
    chain hides under PE-bound work.
  - Host: input slicing/layout, final O(N) reductions (log/mean) only.
"""

import contextlib

import ml_dtypes
import numpy as np

import concourse.bass as bass
import concourse.mybir as mybir
import concourse.tile as tile
from concourse.bass import ds, ts
from concourse.bass_utils import run_bass_kernel_spmd
from concourse.masks import make_identity

F32 = mybir.dt.float32
BF16 = mybir.dt.bfloat16
AF = mybir.ActivationFunctionType
ALU = mybir.AluOpType
AX = mybir.AxisListType

NCORES = 8
H = 768
HS = H // 128          # 6 h-subtiles
NL = 1024              # rows per core
RT = NL // 128         # 8 row tiles
NGLOB = NCORES * NL    # 8192
POOLN = 1024           # token pool size
SCAP = 128             # per-core sampled-row capacity
KTOP = 64
TEMP = 0.1
ITEMP = 1.0 / TEMP
LIM = 0.25
MW = 0.1
EPS = 1e-5
MINV = -1.0e30
P = 128


def _split_drain_waits(nc):
    """This container's walrus accepts at most ONE sync wait per instruction,
    while Tile's add_semaphores pass attaches several. Move extra waits onto
    NoOp instructions inserted right before (same engine, serial execution,
    so blocking semantics are preserved)."""
    for f in nc.m.functions:
        for bb in f.blocks:
            out = []
            changed = False
            for inst in bb.instructions:
                si = inst.sync_info
                if si is not None and len(si.on_wait) > 1:
                    waits = list(si.on_wait)
                    for k, w in enumerate(waits[:-1]):
                        nop = mybir.InstNoOp(name=f"{inst.name}-w{k}", ins=[], outs=[])
                        nop.engine = inst.engine
                        nop.sync_info = mybir.SyncInfo(on_update=[], on_wait=[w])
                        out.append(nop)
                        nc.register_instruction(nop, overwrite=True)
                    si.on_wait = [waits[-1]]
                    changed = True
                out.append(inst)
            if changed:
                bb.instructions = out


def _ln_standardize(nc, pool, src_ap, dst_ap, eps_t):
    """dst = (src - mean)/sqrt(var+eps) rowwise over H ([128, H] tiles).
    dst may be bf16 (fused cast). Uses bn_stats (Welford) for mean/var."""
    sg = src_ap.rearrange("p (n s) -> p n s", s=256)
    stats = pool.tile([P, 3, 6], F32, tag="bnst")
    for g in range(3):
        nc.vector.bn_stats(out=stats[:, g, :], in_=sg[:, g, :])
    mv = pool.tile([P, 2], F32, tag="bnmv")
    nc.vector.bn_aggr(out=mv, in_=stats)
    sd = pool.tile([P, 1], F32, tag="ln_sd")
    nc.scalar.activation(sd, mv[:, 1:2], AF.Sqrt, bias=eps_t)
    rr = pool.tile([P, 1], F32, tag="ln_rr")
    nc.vector.reciprocal(rr, sd)
    nc.vector.tensor_scalar(
        dst_ap, src_ap, mv[:, 0:1], rr, op0=ALU.subtract, op1=ALU.mult
    )


def _l2norm_rows(nc, pool, v_ap, out_ap=None):
    """l2-normalize rows of v_ap [128, H]; result lands in out_ap (may be
    bf16 -> fused cast) or in place."""
    sq = pool.tile([P, H], F32, tag="ln_sq")
    ss = pool.tile([P, 1], F32, tag="ln_ss")
    nc.scalar.activation(sq, v_ap, AF.Square, accum_out=ss)
    nrm = pool.tile([P, 1], F32, tag="ln_m")
    nc.scalar.activation(nrm, ss, AF.Sqrt)
    nc.vector.tensor_scalar_max(nrm, nrm, 1e-12)
    rr = pool.tile([P, 1], F32, tag="ln_rr")
    nc.vector.reciprocal(rr, nrm)
    nc.vector.tensor_scalar_mul(
        out_ap if out_ap is not None else v_ap, v_ap, rr
    )


def _build_nc(ln_trivial):
    nc = bass.Bass(num_devices=NCORES, name="comp_align")

    # ---------------- DRAM I/O ----------------
    def inp(name, shape, dt=F32):
        return nc.dram_tensor(name, shape, dt, kind="ExternalInput")

    x_d = inp("x", [NL, H])
    na_d = inp("na", [NL, H], BF16)
    nb_d = inp("nb_", [NL, H], BF16)
    w_d = {k: inp(k, [H, H], BF16)
           for k in ("w1s", "w2s", "w1b", "w2b", "w1c", "w2c")}
    b1_d = {k: inp(k, [P, HS]) for k in ("b1s", "b1b", "b1c")}
    b2_d = {k: inp(k, [P, H]) for k in ("b2s", "b2b", "b2c")}
    g_d = {k: inp(k, [P, HS]) for k in ("gs", "bts", "gb", "btb", "gc", "btc")}
    te_d = inp("te", [POOLN, H])
    tenT_d = inp("tenT", [H, POOLN])
    ohT_d = inp("ohT", [NL, SCAP])
    val_d = inp("val", [SCAP, 1])

    alig_o = nc.dram_tensor("alig_o", [NL, H], F32, kind="ExternalOutput")
    scal_o = nc.dram_tensor("scal_o", [NL, H], F32, kind="ExternalOutput")
    bias_o = nc.dram_tensor("bias_o", [NL, H], F32, kind="ExternalOutput")
    dcand_o = nc.dram_tensor("dcand_o", [P, RT, NCORES], F32, kind="ExternalOutput")
    rsum_o = nc.dram_tensor("rsum_o", [P, RT], F32, kind="ExternalOutput")
    csum_o = nc.dram_tensor("csum_o", [2 * NCORES, 512], F32, kind="ExternalOutput")
    reg_o = nc.dram_tensor("reg_o", [P, 2], F32, kind="ExternalOutput")
    afm_o = nc.dram_tensor("afm_o", [SCAP, H], F32, kind="ExternalOutput")
    prot_o = nc.dram_tensor("prot_o", [SCAP, H], F32, kind="ExternalOutput")

    with tile.TileContext(nc) as tc:
        with contextlib.ExitStack() as ctx:
            # ------------ long-lived pools ------------
            const = ctx.enter_context(tc.tile_pool(name="const", bufs=1))
            longp = ctx.enter_context(tc.tile_pool(name="longp", bufs=1))
            accp = ctx.enter_context(tc.tile_pool(name="accp", bufs=1))
            dram = ctx.enter_context(tc.tile_pool(name="dram", bufs=1, space="DRAM"))

            identity = const.tile([P, P], F32)
            make_identity(nc, identity)
            ones_col = const.tile([P, 1], F32)
            nc.vector.memset(ones_col, 1.0)
            eps_t = const.tile([P, 1], F32)
            nc.vector.memset(eps_t, EPS)
            b1_sb = {k: const.tile([P, HS], F32, name=f"sb_{k}") for k in b1_d}
            for k in b1_d:
                nc.sync.dma_start(b1_sb[k][:], b1_d[k][:])
            b2_sb = {k: const.tile([P, H], F32, name=f"sb_{k}") for k in b2_d}
            for k in b2_d:
                nc.sync.dma_start(b2_sb[k][:], b2_d[k][:])
            g_sb = {k: const.tile([P, HS], F32, name=f"sb_{k}") for k in g_d}
            for k in g_d:
                nc.sync.dma_start(g_sb[k][:], g_d[k][:])
            val_sb = const.tile([SCAP, 1], F32)
            nc.sync.dma_start(val_sb[:], val_d[:])

            # x shard stays resident; vaT persists into the logits phase
            x_sb = longp.tile([P, RT, H], F32)
            nc.sync.dma_start(x_sb[:], x_d.rearrange("(rt p) h -> p rt h", p=P))
            vaT = longp.tile([P, HS, NL], BF16)

            # accumulators
            rs_parts = accp.tile([P, RT, 16], F32)
            dcand = accp.tile([P, RT, NCORES], F32)
            racc_s = accp.tile([P, 16], F32)
            racc_b = accp.tile([P, 16], F32)

            # phase-D pools live at top level so logits tiles do not
            # wait for the head-phase pools' SBUF space to free
            kxn = ctx.enter_context(tc.tile_pool(name="kxn", bufs=2))
            expp = ctx.enter_context(tc.tile_pool(name="expp", bufs=3))
            caccp = ctx.enter_context(tc.tile_pool(name="caccp", bufs=2))
            csump = ctx.enter_context(tc.tile_pool(name="csump", bufs=2))
            junkp = ctx.enter_context(tc.tile_pool(name="junkp", bufs=1))
            psd = ctx.enter_context(tc.tile_pool(name="ps_d", bufs=3, space="PSUM"))
            psd1 = ctx.enter_context(
                tc.tile_pool(name="ps_d1", bufs=1, space="PSUM")
            )

            # DRAM scratch
            ag_in = [dram.tile([HS, P, 512], BF16, name=f"ag_in{h}")
                     for h in range(2)]
            ag_out = [
                dram.tile([NCORES, HS, P, 512], BF16, addr_space="Shared",
                          name=f"ag_out{h}")
                for h in range(2)
            ]
            alig_d = dram.tile([NL, H], F32)
            xt_d = {k: dram.tile([NL, H], BF16, name=f"xt_{k}")
                    for k in ("xn", "xa", "xb", "va", "vb")}

            def std_store_transpose(noise_dram, xdram, dstT, lnp,
                                    ln_g, ln_b):
                """Standardize x (+ optional noise) per row tile, cast to
                bf16, store to DRAM, then two half DMA-crossbar transposes
                into dstT [128, HS, NL] bf16 (halves so downstream GEMMs can
                start after 4 row tiles)."""
                for rt in range(RT):
                    if noise_dram is not None:
                        nz = lnp.tile([P, H], BF16, tag="nz")
                        nc.sync.dma_start(nz[:], noise_dram[ds(rt * P, P), :])
                        xa_t = lnp.tile([P, H], F32, tag="xa")
                        nc.vector.tensor_add(xa_t, x_sb[:, rt, :], nz)
                        src_ap = xa_t[:]
                    else:
                        src_ap = x_sb[:, rt, :]
                    xab = xabp.tile([P, H], BF16, tag="xab")
                    _ln_standardize(nc, lnp, src_ap, xab[:], eps_t)
                    nc.sync.dma_start(xdram[ds(rt * P, P), :], xab[:])
                    if rt in (RT // 2 - 1, RT - 1):
                        hb = 0 if rt < RT // 2 else 1
                        nc.sync.dma_start_transpose(
                            dstT[:, :, ts(hb, 512)], xdram[ds(hb * 512, 512), :]
                        )
                if not ln_trivial:
                    for hs in range(HS):
                        nc.vector.tensor_scalar(
                            dstT[:, hs, :], dstT[:, hs, :],
                            ln_g[:, hs : hs + 1], ln_b[:, hs : hs + 1],
                            op0=ALU.mult, op1=ALU.add,
                        )

            with contextlib.ExitStack() as hctx:
                wall = hctx.enter_context(tc.tile_pool(name="wall", bufs=4))
                bigp = hctx.enter_context(tc.tile_pool(name="bigp", bufs=4))
                lnp = hctx.enter_context(tc.tile_pool(name="lnp", bufs=2))
                epp = hctx.enter_context(tc.tile_pool(name="epp", bufs=2))
                xabp = hctx.enter_context(tc.tile_pool(name="xabp", bufs=4))
                psh = hctx.enter_context(
                    tc.tile_pool(name="psh", bufs=4, space="PSUM")
                )

                # preload every weight up front (keeps the DMA queues free
                # while the AllGather runs)
                w_sb = {}
                for k in ("w1c", "w2c", "w1s", "w1b", "w2s", "w2b"):
                    w_sb[k] = wall.tile([P, HS, H], BF16, tag="w", name=f"w_{k}")
                    nc.sync.dma_start(
                        w_sb[k][:], w_d[k].rearrange("(ks p) o -> p ks o", p=P)
                    )

                def first_gemm(w1_sb, lnT, g1T, b1, psum):
                    for mt in range(HS):
                        for nb in range(2):
                            ps = psum.tile([P, 512], F32, tag="gemm")
                            for ks in range(HS):
                                nc.tensor.matmul(
                                    ps,
                                    w1_sb[:, ks, ts(mt, P)],
                                    lnT[:, ks, ts(nb, 512)],
                                    start=(ks == 0),
                                    stop=(ks == HS - 1),
                                )
                            nc.scalar.activation(
                                g1T[:, mt, ts(nb, 512)], ps, AF.Gelu,
                                bias=b1[:, mt : mt + 1],
                            )

                def con_second(which, g1T, ag):
                    """second GEMM of the contrastive head; l2-normalized bf16
                    rows land in xt_d[which]; at each half boundary kick the
                    DMA transpose (and, for vb, the AllGather half)."""
                    for rt in range(RT):
                        vr = lnp.tile([P, H], F32, tag="xa")
                        for nb2, width in ((0, 512), (1, 256)):
                            ps = psh.tile([P, 512], F32, tag="gemm")
                            pw = ps[:, :width]
                            for ks in range(HS):
                                nc.tensor.matmul(
                                    pw,
                                    g1T[:, ks, ts(rt, P)],
                                    w_sb["w2c"][:, ks, ds(nb2 * 512, width)],
                                    start=(ks == 0),
                                    stop=(ks == HS - 1),
                                )
                            nc.vector.tensor_add(
                                vr[:, ds(nb2 * 512, width)], pw,
                                b2_sb["b2c"][:, ds(nb2 * 512, width)],
                            )
                        vb_ = xabp.tile([P, H], BF16, tag="xab")
                        _l2norm_rows(nc, lnp, vr[:], vb_[:])
                        nc.sync.dma_start(xt_d[which][ds(rt * P, P), :], vb_[:])
                        if rt in (RT // 2 - 1, RT - 1):
                            hb = 0 if rt < RT // 2 else 1
                            vT = vaT if which == "va" else vbT
                            nc.sync.dma_start_transpose(
                                vT[:, :, ts(hb, 512)],
                                xt_d[which][ds(hb * 512, 512), :],
                            )
                            if ag:
                                nc.sync.dma_start(
                                    ag_in[hb][:].rearrange("ks p i -> p ks i"),
                                    vT[:, :, ts(hb, 512)],
                                )
                                nc.gpsimd.collective_compute(
                                    "AllGather",
                                    ALU.bypass,
                                    replica_groups=[list(range(NCORES))],
                                    ins=[ag_in[hb][:].opt()],
                                    outs=[ag_out[hb][:].opt()],
                                )

                # ---- all LN chains first (DVE/ACT/DMA work, PE-free) ----
                xbnT = bigp.tile([P, HS, NL], BF16, tag="bigT", name="xbnT")
                std_store_transpose(nb_d, xt_d["xb"], xbnT, lnp,
                                    g_sb["gc"], g_sb["btc"])
                xanT = bigp.tile([P, HS, NL], BF16, tag="bigT", name="xanT")
                std_store_transpose(na_d, xt_d["xa"], xanT, lnp,
                                    g_sb["gc"], g_sb["btc"])
                xnT = bigp.tile([P, HS, NL], BF16, tag="bigT", name="xnT")
                std_store_transpose(None, xt_d["xn"], xnT, lnp,
                                    g_sb["gs"], g_sb["bts"])
                if ln_trivial:
                    lnT = {"s": xnT, "b": xnT}
                else:
                    lnT = {"s": xnT,
                           "b": bigp.tile([P, HS, NL], BF16, tag="bigT",
                                          name="lnT_b")}
                    std_store_transpose(None, xt_d["xn"], lnT["b"], lnp,
                                        g_sb["gb"], g_sb["btb"])

                # ---- view b head -> split AllGather (earliest possible) ----
                g1T_vb = bigp.tile([P, HS, NL], BF16, tag="bigT", name="g1T_vb")
                first_gemm(w_sb["w1c"], xbnT, g1T_vb, b1_sb["b1c"], psh)
                vbT = bigp.tile([P, HS, NL], BF16, tag="bigT", name="vbT")
                con_second("vb", g1T_vb, ag=True)

                # ---- view a head (fills PE while the AllGather flies) ----
                g1T_va = bigp.tile([P, HS, NL], BF16, tag="bigT", name="g1T_va")
                first_gemm(w_sb["w1c"], xanT, g1T_va, b1_sb["b1c"], psh)
                con_second("va", g1T_va, ag=False)

                # ---- scale & bias heads ----
                g1T_s = bigp.tile([P, HS, NL], BF16, tag="bigT", name="g1T_s")
                first_gemm(w_sb["w1s"], lnT["s"], g1T_s, b1_sb["b1s"], psh)
                g1T_b = bigp.tile([P, HS, NL], BF16, tag="bigT", name="g1T_b")
                first_gemm(w_sb["w1b"], lnT["b"], g1T_b, b1_sb["b1b"], psh)

                for rt in range(RT):
                    for nb2, width in ((0, 512), (1, 256)):
                        sl = ds(nb2 * 512, width)
                        col = rt * 2 + nb2
                        # scale head tile
                        ps_s = psh.tile([P, 512], F32, tag="gemm")
                        for ks in range(HS):
                            nc.tensor.matmul(
                                ps_s[:, :width],
                                g1T_s[:, ks, ts(rt, P)],
                                w_sb["w2s"][:, ks, sl],
                                start=(ks == 0), stop=(ks == HS - 1),
                            )
                        h1 = epp.tile([P, 512], F32, tag="ep_h1")
                        nc.vector.tensor_add(
                            h1[:, :width], ps_s[:, :width], b2_sb["b2s"][:, sl]
                        )
                        th_s = epp.tile([P, 512], F32, tag="ep_th")
                        nc.scalar.activation(th_s[:, :width], h1[:, :width], AF.Tanh)
                        sqs = epp.tile([P, 512], F32, tag="ep_h1")
                        nc.scalar.activation(
                            sqs[:, :width], th_s[:, :width], AF.Square,
                            accum_out=racc_s[:, col : col + 1],
                        )
                        sc = epp.tile([P, 512], F32, tag="ep_sc")
                        nc.vector.tensor_scalar(
                            sc[:, :width], th_s[:, :width],
                            LIM, 1.0, op0=ALU.mult, op1=ALU.add,
                        )
                        nc.sync.dma_start(scal_o[ds(rt * P, P), sl], sc[:, :width])
                        # bias head tile
                        ps_b = psh.tile([P, 512], F32, tag="gemm")
                        for ks in range(HS):
                            nc.tensor.matmul(
                                ps_b[:, :width],
                                g1T_b[:, ks, ts(rt, P)],
                                w_sb["w2b"][:, ks, sl],
                                start=(ks == 0), stop=(ks == HS - 1),
                            )
                        h2 = epp.tile([P, 512], F32, tag="ep_h1")
                        nc.vector.tensor_add(
                            h2[:, :width], ps_b[:, :width], b2_sb["b2b"][:, sl]
                        )
                        th_b = epp.tile([P, 512], F32, tag="ep_th")
                        nc.scalar.activation(th_b[:, :width], h2[:, :width], AF.Tanh)
                        sqb = epp.tile([P, 512], F32, tag="ep_h1")
                        nc.scalar.activation(
                            sqb[:, :width], th_b[:, :width], AF.Square,
                            accum_out=racc_b[:, col : col + 1],
                        )
                        bi = epp.tile([P, 512], F32, tag="ep_bi")
                        nc.vector.tensor_scalar_mul(bi[:, :width], th_b[:, :width], LIM)
                        nc.sync.dma_start(bias_o[ds(rt * P, P), sl], bi[:, :width])
                        # aligned = scale * x + bias
                        al = epp.tile([P, 512], F32, tag="ep_al")
                        nc.vector.tensor_mul(
                            al[:, :width], sc[:, :width], x_sb[:, rt, sl]
                        )
                        nc.vector.tensor_add(al[:, :width], al[:, :width], bi[:, :width])
                        nc.sync.dma_start(alig_o[ds(rt * P, P), sl], al[:, :width])
                        nc.sync.dma_start(alig_d[ds(rt * P, P), sl], al[:, :width])

                rs1 = lnp.tile([P, 1], F32, tag="reg1")
                nc.vector.reduce_sum(out=rs1, in_=racc_s[:], axis=AX.X)
                nc.sync.dma_start(reg_o[:, 0:1], rs1[:])
                rs2 = lnp.tile([P, 1], F32, tag="reg2")
                nc.vector.reduce_sum(out=rs2, in_=racc_b[:], axis=AX.X)
                nc.sync.dma_start(reg_o[:, 1:2], rs2[:])


            # =========================================================
            # Phases C + D together: the serial token-distribution chain
            # (C) hides under the PE-bound logits phase (D).
            # =========================================================
            with contextlib.ExitStack() as dctx:
                tokp = dctx.enter_context(tc.tile_pool(name="tokp", bufs=1))
                tok2 = dctx.enter_context(tc.tile_pool(name="tok2", bufs=2))
                psc = dctx.enter_context(
                    tc.tile_pool(name="ps_c", bufs=2, space="PSUM")
                )
                pstc = dctx.enter_context(
                    tc.tile_pool(name="pst_c", bufs=1, space="PSUM")
                )

                # ---------------- Phase C ----------------
                ohT_sb = tokp.tile([P, RT, SCAP], F32)
                nc.sync.dma_start(
                    ohT_sb[:], ohT_d.rearrange("(ks p) s -> p ks s", p=P)
                )
                af = tokp.tile([P, H], F32)
                for nb2, width in ((0, 512), (1, 256)):
                    ps = psc.tile([P, 512], F32, tag="cgem")
                    pw = ps[:, :width]
                    for ks in range(RT):
                        rhs_t = tok2.tile([P, 512], F32, tag="argh")
                        nc.sync.dma_start(
                            rhs_t[:, :width],
                            alig_d[ds(ks * P, P), ds(nb2 * 512, width)],
                        )
                        nc.tensor.matmul(
                            pw,
                            ohT_sb[:, ks, :],
                            rhs_t[:, :width],
                            start=(ks == 0),
                            stop=(ks == RT - 1),
                        )
                    nc.scalar.copy(af[:, ds(nb2 * 512, width)], pw)

                afn = tokp.tile([P, H], F32)
                nc.vector.tensor_copy(afn[:], af[:])
                _l2norm_rows(nc, tok2, afn[:])

                afnT = tokp.tile([P, HS, SCAP], F32)
                for hs in range(HS):
                    pt = pstc.tile([P, P], F32, tag="tpc")
                    nc.tensor.transpose(pt, afn[:, ts(hs, P)], identity)
                    nc.scalar.copy(afnT[:, hs, :], pt)

                tenT_sb = tokp.tile([P, HS, POOLN], F32)
                nc.sync.dma_start(
                    tenT_sb[:], tenT_d.rearrange("(ks p) j -> p ks j", p=P)
                )
                tl = tokp.tile([P, POOLN], F32)
                for half in range(2):
                    ps = psc.tile([P, 512], F32, tag="cgem")
                    for ks in range(HS):
                        nc.tensor.matmul(
                            ps,
                            afnT[:, ks, :],
                            tenT_sb[:, ks, ts(half, 512)],
                            start=(ks == 0),
                            stop=(ks == HS - 1),
                        )
                    nc.scalar.copy(tl[:, ts(half, 512)], ps)

                # top-64 mask via 8x (max8 + match_replace)
                work = tokp.tile([P, POOLN], F32)
                nc.vector.tensor_copy(work[:], tl[:])
                maxb = tokp.tile([P, 8], F32)
                for _ in range(KTOP // 8):
                    nc.vector.max(out=maxb[:], in_=work[:])
                    nc.vector.match_replace(
                        out=work[:], in_to_replace=maxb[:], in_values=work[:],
                        imm_value=MINV,
                    )
                mask = tokp.tile([P, POOLN], F32)
                nc.vector.tensor_sub(mask[:], tl[:], work[:])
                nc.vector.tensor_scalar_min(mask[:], mask[:], 1.0)

                ew = tokp.tile([P, POOLN], F32)
                nc.scalar.activation(ew[:], tl[:], AF.Exp)
                nc.vector.tensor_mul(ew[:], ew[:], mask[:])
                zz = tokp.tile([P, 1], F32)
                nc.vector.reduce_sum(out=zz, in_=ew[:], axis=AX.X)
                zr = tokp.tile([P, 1], F32)
                nc.vector.reciprocal(zr, zz)

                eT = tokp.tile([P, RT, SCAP], F32)
                for js in range(RT):
                    pt = pstc.tile([P, P], F32, tag="tpc")
                    nc.tensor.transpose(pt, ew[:, ts(js, P)], identity)
                    nc.scalar.copy(eT[:, js, :], pt)

                te_sb = tokp.tile([P, RT, H], F32)
                nc.sync.dma_start(
                    te_sb[:], te_d.rearrange("(js p) o -> p js o", p=P)
                )
                prot = tokp.tile([P, H], F32)
                for nb2, width in ((0, 512), (1, 256)):
                    ps = psc.tile([P, 512], F32, tag="cgem")
                    pw = ps[:, :width]
                    for js in range(RT):
                        nc.tensor.matmul(
                            pw,
                            eT[:, js, :],
                            te_sb[:, js, ds(nb2 * 512, width)],
                            start=(js == 0),
                            stop=(js == RT - 1),
                        )
                    nc.vector.tensor_scalar_mul(prot[:, ds(nb2 * 512, width)], pw, zr)

                nc.vector.tensor_scalar_mul(af[:], af[:], val_sb)
                nc.vector.tensor_scalar_mul(prot[:], prot[:], val_sb)
                nc.sync.dma_start(afm_o[:, :], af[:])
                nc.sync.dma_start(prot_o[:, :], prot[:])

                # ---------------- Phase D ----------------
                for half in range(2):
                    for c2 in range(NCORES):
                        nbi = c2 * 2 + half
                        rhs_t = kxn.tile([P, HS, 512], BF16, tag="kxn")
                        nc.sync.dma_start(
                            rhs_t[:],
                            ag_out[half][c2].rearrange("hs p i -> p hs i"),
                        )
                        cacc = caccp.tile([P, 512], F32, tag="cacc")
                        for rt in range(RT):
                            ps = psd.tile([P, 512], F32, tag="lg")
                            for ks in range(HS):
                                nc.tensor.matmul(
                                    ps,
                                    vaT[:, ks, ts(rt, P)],
                                    rhs_t[:, ks, :],
                                    start=(ks == 0),
                                    stop=(ks == HS - 1),
                                )
                            if half == rt // 4:
                                off = (rt % 4) * P
                                junk = junkp.tile([P, P], F32, tag="junk")
                                nc.vector.tensor_mul(
                                    junk[:], ps[:, ds(off, P)], identity[:]
                                )
                                nc.vector.reduce_sum(
                                    out=dcand[:, rt, c2 : c2 + 1],
                                    in_=junk[:],
                                    axis=AX.X,
                                )
                            et = expp.tile([P, 512], F32, tag="exp")
                            nc.scalar.activation(
                                et[:], ps, AF.Exp, scale=ITEMP,
                                accum_out=rs_parts[:, rt, nbi : nbi + 1],
                            )
                            if rt == 0:
                                nc.vector.tensor_copy(cacc[:], et[:])
                            else:
                                nc.vector.tensor_add(cacc[:], cacc[:], et[:])
                        cps = psd1.tile([1, 512], F32, tag="cs")
                        nc.tensor.matmul(
                            cps, ones_col[:, :], cacc[:], start=True, stop=True
                        )
                        csb = csump.tile([1, 512], F32, tag="csb")
                        nc.scalar.copy(csb[:], cps)
                        nc.sync.dma_start(csum_o[nbi : nbi + 1, :], csb[:, :])

                rsf = csump.tile([P, RT], F32, tag="rsf")
                nc.vector.reduce_sum(out=rsf, in_=rs_parts[:], axis=AX.X)
                nc.sync.dma_start(rsum_o[:, :], rsf[:])
                nc.sync.dma_start(dcand_o[:, :, :], dcand[:])

    _split_drain_waits(nc)
    return nc


_CACHE = {}


def _get_nc(ln_trivial=True):
    key = ("nc", ln_trivial)
    if key not in _CACHE:
        _CACHE[key] = _build_nc(ln_trivial)
    return _CACHE[key]


def _prep_in_maps(inputs):
    f32 = np.float32
    lat = np.ascontiguousarray(np.asarray(inputs["latent"], f32)).reshape(NGLOB, H)
    na = np.ascontiguousarray(np.asarray(inputs["noise_a"], f32))
    nb = np.ascontiguousarray(np.asarray(inputs["noise_b"], f32))
    tem = np.asarray(inputs["token_embedding_matrix"], f32)
    samp = np.asarray(inputs["samp_idx"]).astype(np.int64)
    toki = np.asarray(inputs["tok_idx"]).astype(np.int64)

    te = np.ascontiguousarray(tem[toki])                      # [POOLN, H]
    ten = te / np.maximum(
        np.linalg.norm(te, axis=-1, keepdims=True), 1e-12
    )
    tenT = np.ascontiguousarray((ten * ITEMP).T)              # [H, POOLN] (x 1/TEMP)

    ln_trivial = all(
        np.all(np.asarray(inputs[k]) == 1.0)
        for k in ("scale_ln_g", "bias_ln_g", "con_ln_g")
    ) and all(
        np.all(np.asarray(inputs[k]) == 0.0)
        for k in ("scale_ln_b", "bias_ln_b", "con_ln_b")
    )

    def pk(v):  # [H] vector -> [P, HS] per-partition layout (h = ks*P + p)
        return np.ascontiguousarray(np.asarray(v, f32).reshape(HS, P).T)

    def bc(v):  # [H] vector -> [P, H] broadcast
        return np.ascontiguousarray(
            np.broadcast_to(np.asarray(v, f32)[None, :], (P, H))
        )

    bf16 = ml_dtypes.bfloat16
    common = {
        "w1s": np.ascontiguousarray(np.asarray(inputs["scale_w1"], bf16)),
        "w2s": np.ascontiguousarray(np.asarray(inputs["scale_w2"], bf16)),
        "w1b": np.ascontiguousarray(np.asarray(inputs["bias_w1"], bf16)),
        "w2b": np.ascontiguousarray(np.asarray(inputs["bias_w2"], bf16)),
        "w1c": np.ascontiguousarray(np.asarray(inputs["con_w1"], bf16)),
        "w2c": np.ascontiguousarray(np.asarray(inputs["con_w2"], bf16)),
        "b1s": pk(inputs["scale_b1"]), "b1b": pk(inputs["bias_b1"]),
        "b1c": pk(inputs["con_b1"]),
        "b2s": bc(inputs["scale_b2"]), "b2b": bc(inputs["bias_b2"]),
        "b2c": bc(inputs["con_b2"]),
        "gs": pk(inputs["scale_ln_g"]), "bts": pk(inputs["scale_ln_b"]),
        "gb": pk(inputs["bias_ln_g"]), "btb": pk(inputs["bias_ln_b"]),
        "gc": pk(inputs["con_ln_g"]), "btc": pk(inputs["con_ln_b"]),
        "te": te,
        "tenT": tenT,
    }

    # sampled-row ownership
    slots = [[] for _ in range(NCORES)]
    for g in samp:
        c, loc = int(g) // NL, int(g) % NL
        slots[c].append(loc)
    in_maps = []
    for c in range(NCORES):
        locs = slots[c]
        assert len(locs) <= SCAP, f"core {c} owns {len(locs)} sampled rows > {SCAP}"
        ohT = np.zeros((NL, SCAP), f32)
        val = np.zeros((SCAP, 1), f32)
        for s, loc in enumerate(locs):
            ohT[loc, s] = 1.0
            val[s, 0] = 1.0
        for s in range(len(locs), SCAP):
            ohT[0, s] = 1.0  # dup row 0, masked out by val
        m = dict(common)
        m["x"] = np.ascontiguousarray(lat[c * NL : (c + 1) * NL])
        m["na"] = np.ascontiguousarray(na[c * NL : (c + 1) * NL].astype(bf16))
        m["nb_"] = np.ascontiguousarray(nb[c * NL : (c + 1) * NL].astype(bf16))
        m["ohT"] = ohT
        m["val"] = val
        in_maps.append(m)
    return in_maps, te, ln_trivial


def _finish(results, te):
    f64 = np.float64
    aligned = np.concatenate([r["alig_o"] for r in results], 0).reshape(8, 1024, H)
    scale = np.concatenate([r["scal_o"] for r in results], 0).reshape(8, 1024, H)
    biast = np.concatenate([r["bias_o"] for r in results], 0).reshape(8, 1024, H)

    # contrastive loss
    diag = np.concatenate(
        [results[c]["dcand_o"][:, :, c].T.reshape(NL) for c in range(NCORES)]
    ) * ITEMP
    rowsum = np.concatenate(
        [results[c]["rsum_o"].T.reshape(NL) for c in range(NCORES)]
    )
    colsum = np.sum([r["csum_o"].reshape(NGLOB) for r in results], axis=0)
    la = -np.mean(diag - np.log(rowsum))
    lb = -np.mean(diag - np.log(colsum))
    con_loss = 0.5 * (la + lb)

    # comp reg
    reg = np.sum([r["reg_o"].sum(0) for r in results], axis=0)
    comp_reg = (LIM * LIM) * (reg[0] + reg[1]) / (NGLOB * H)

    # token distribution
    afm = np.concatenate([r["afm_o"] for r in results], 0).astype(f64)
    prm = np.concatenate([r["prot_o"] for r in results], 0).astype(f64)
    S = 256
    proto_loss = np.sum((afm - prm) ** 2) / (S * H)
    s1 = afm.sum(0)
    s2 = (afm * afm).sum(0)
    af_mean = s1 / S
    af_var = np.maximum(s2 / S - af_mean**2, 0.0)
    af_std = np.sqrt(af_var)
    te64 = te.astype(f64)
    te_mean = te64.mean(0)
    te_std = te64.std(0)
    moment = np.mean((af_mean - te_mean) ** 2) + np.mean((af_std - te_std) ** 2)
    token_dist = proto_loss + MW * moment

    return (
        aligned.astype(np.float32),
        np.array(con_loss, np.float32),
        np.array(token_dist, np.float32),
        np.array(comp_reg, np.float32),
        scale.astype(np.float32),
        biast.astype(np.float32),
    )


def kernel(**inputs):
    in_maps, te, ln_trivial = _prep_in_maps(inputs)
    nc = _get_nc(ln_trivial)
    res = run_bass_kernel_spmd(nc, in_maps, core_ids=list(range(NCORES)))
    return _finish(res.results, te)
